# revision 24
# baseline (speedup 1.0000x reference)
"""Trainium2 Bass kernel for ChunkGatedAttentionUnit.

Sharding: 2 batch groups x 4-way tensor parallel on expanded dim D.
Core c handles batch b=c//4, D-slice j=c%4 (DL=512 columns).

Per core:
  - projections: q^T/k^T slices (weight-stationary, s-moving) staged to DRAM
    and all-gathered within the 4-core batch group (NSPL s-blocks, pipelined
    behind the projection blocks); v/g slices (x-stationary, w-moving) kept
    in SBUF / DRAM fp16.
  - chunk loop (cs=128, 32 chunks): intra-chunk causal softmax attention +
    inter-chunk linear attention against an fp16 state master that matmuls
    read directly (updated once per chunk PAIR via psum accumulation; odd
    chunks get an exact rank-128 correction (q_i k_{i-1}^T) v_{i-1} whose
    score matrix rides the same stationary tiles as the softmax scores).
  - k natural tiles come from DMA-transpose (XBAR), not the PE.
  - state master adds: Activation stages psum->fp16, Pool accumulates, so
    neither the DVE softmax chain nor the PE ever wait on them.
  - out-projection partials reduce-scattered in RSPL blocks.
"""

import sys
import math

sys.path.insert(0, "/opt/trn_rl_repo")

import numpy as np
import concourse.bass as bass
from concourse import mybir
from concourse import masks
from concourse.tile import TileContext
from concourse.bass_utils import run_bass_kernel_spmd

FP32 = mybir.dt.float32
FP16 = mybir.dt.float16
BF16 = mybir.dt.bfloat16
ACT_COPY = mybir.ActivationFunctionType.Copy
ACT_EXP = mybir.ActivationFunctionType.Exp
ACT_SIGMOID = mybir.ActivationFunctionType.Sigmoid

B, S, H, D = 2, 4096, 1024, 2048
CS = 128
NCORE = 8
GROUP = 4


def split_excess_waits(nc, limit=1):
    """This walrus build rejects instructions with >limit sync waits; move
    excess waits onto standalone event-semaphore instructions just before."""
    n = 0
    for f in nc.m.functions:
        for bb in f.blocks:
            new_insts = []
            for inst in bb.instructions:
                si = inst.sync_info
                if si is not None and si.on_wait and len(si.on_wait) > limit:
                    waits = list(si.on_wait)
                    excess, keep = waits[:-limit], waits[-limit:]
                    for j in range(0, len(excess), limit):
                        n += 1
                        es = mybir.InstEventSemaphore(
                            name=f"{inst.name}_wsplit{n}",
                            ins=[],
                            outs=[],
                            sync_info=mybir.SyncInfo(
                                on_wait=excess[j : j + limit], on_update=[]
                            ),
                        )
                        es.engine = inst.engine
                        new_insts.append(es)
                    si.on_wait = keep
                new_insts.append(inst)
            bb.instructions = new_insts
    return n


def build(S=S, H=H, D=D, cs=CS, ncore=NCORE, group=GROUP, split_waits=True,
          nspl=2, rspl=4, with_bias=True):
    """Emit the SPMD Tile program. Returns nc."""
    DL = D // group
    nH = H // 128
    nC = S // cs
    nDT = D // 128
    nDL = DL // 128
    NSPL = nspl
    Sn = S // NSPL            # rows per AG block
    nSSb = Sn // 512          # 512-col s-slices per AG block
    cpb = nC // NSPL          # chunks per AG block
    RSPL = rspl
    Sn2 = S // RSPL           # rows per RS block
    cpr = nC // RSPL          # chunks per RS block
    Sg2 = Sn2 // group        # RS output rows per rank per block
    scale = 1.0 / math.sqrt(D)
    groups = [list(range(g * group, (g + 1) * group))
              for g in range(ncore // group)]

    nc = bass.Bass("TRN2", target_bir_lowering=False, debug=False,
                   num_devices=ncore)

    # ---- I/O (X^T pre-transposed on host) ----
    xt_in = nc.dram_tensor("xt", [H, S], BF16, kind="ExternalInput")
    wq_in = nc.dram_tensor("wq", [H, DL], BF16, kind="ExternalInput")
    wk_in = nc.dram_tensor("wk", [H, DL], BF16, kind="ExternalInput")
    wv_in = nc.dram_tensor("wv", [H, DL], BF16, kind="ExternalInput")
    wg_in = nc.dram_tensor("wg", [H, DL], BF16, kind="ExternalInput")
    wo_in = nc.dram_tensor("wo", [DL, H], BF16, kind="ExternalInput")
    bq_in = nc.dram_tensor("bq", [DL], FP32, kind="ExternalInput")
    bk_in = nc.dram_tensor("bk", [DL], FP32, kind="ExternalInput")
    bv_in = nc.dram_tensor("bv", [DL], FP32, kind="ExternalInput")
    bg_in = nc.dram_tensor("bg", [DL], FP32, kind="ExternalInput")
    bo_in = nc.dram_tensor("bo", [H], FP32, kind="ExternalInput")
    y_out = nc.dram_tensor("y_red", [RSPL, Sn2 // group, H], FP16,
                           kind="ExternalOutput")

    # ---- internal DRAM (per-block: Tile DRAM deps are whole-tensor) ----
    # index 0 = q^T, 1 = k^T
    qk_my = [nc.dram_tensor(f"qk_my{b}", [2, DL, Sn], BF16)
             for b in range(NSPL)]
    qk_ag = [nc.dram_tensor(f"qk_ag{b}", [group, 2, DL, Sn], BF16)
             for b in range(NSPL)]
    part_out = [nc.dram_tensor(f"part_out{r}", [Sn2, H], FP16)
                for r in range(RSPL)]
    rs_out = [nc.dram_tensor(f"rs_out{r}", [Sg2, H], FP16)
              for r in range(RSPL)]

    with TileContext(nc) as tc:
        from contextlib import ExitStack
        with ExitStack() as stack:
            const_pool = stack.enter_context(tc.tile_pool(name="const", bufs=1))
            wo_pool = stack.enter_context(tc.tile_pool(name="wo", bufs=1))
            vres_pool = stack.enter_context(tc.tile_pool(name="vres", bufs=1))
            st_pool = stack.enter_context(tc.tile_pool(name="stmast", bufs=1))
            v_sb = vres_pool.tile([128, nC * DL], BF16, tag="vsb")
            g_sb = vres_pool.tile([128, nC * DL], FP16, tag="gsb")
            master = st_pool.tile([128, nDT * DL], FP16, tag="master")
            nc.gpsimd.memset(master[:], 0.0)

            ident = const_pool.tile([128, 128], BF16, tag="ident")
            masks.make_identity(nc, ident[:])
            causal = const_pool.tile([128, 128], FP32, tag="causal")
            masks.make_causal_mask(nc, causal[:], mask_val=-30000.0)

            bias_row = {}
            bias_vec = {}
            if with_bias:
                ones_row = const_pool.tile([1, 512], BF16, tag="ones")
                nc.gpsimd.memset(ones_row[:], 1.0)
                # q/k biases as per-partition [128, nDL] fp32 vectors
                for name, dram in (("bq", bq_in), ("bk", bk_in)):
                    bv = const_pool.tile([128, nDL], FP32, tag=name + "v")
                    nc.sync.dma_start(
                        out=bv[:],
                        in_=dram.rearrange("(t p) -> p t", p=128))
                    bias_vec[name] = bv
                # v/g/o biases as [1, width] bf16 rows for ones-matmuls
                for name, dram, width in (("bv", bv_in, DL), ("bg", bg_in, DL),
                                          ("bo", bo_in, H)):
                    bf = const_pool.tile([1, width], FP32, tag=name + "f")
                    nc.sync.dma_start(out=bf[:], in_=dram[None, :])
                    bb16 = const_pool.tile([1, width], BF16, tag=name)
                    nc.scalar.activation(bb16[:], bf[:], ACT_COPY)
                    bias_row[name] = bb16

            # Wo resident: [p=dl%128, (dl_tile, H)]
            wo_sb = wo_pool.tile([128, nDL * H], BF16, tag="wo")
            for t in range(nDL):
                nc.sync.dma_start(out=wo_sb[:, t * H:(t + 1) * H],
                                  in_=wo_in[t * 128:(t + 1) * 128, :])

            # ---------- phase 1: projections ----------
            with tc.tile_pool(name="xt", bufs=1) as xt_pool, \
                 tc.tile_pool(name="wsb", bufs=1) as wsb_pool:

                # X^T -> sbuf bf16: [p=h%128, (ht, S)]
                xt_sb = xt_pool.tile([128, nH * S], BF16, tag="xt")
                for ht in range(nH):
                    nc.sync.dma_start(out=xt_sb[:, ht * S:(ht + 1) * S],
                                      in_=xt_in[ht * 128:(ht + 1) * 128, :])

                # weights -> sbuf bf16: [p=h%128, (ht, DL)]
                w_sb = {}
                for name, dram in (("wq", wq_in), ("wk", wk_in),
                                   ("wv", wv_in), ("wg", wg_in)):
                    wt = wsb_pool.tile([128, nH * DL], BF16, tag=name)
                    for ht in range(nH):
                        nc.sync.dma_start(
                            out=wt[:, ht * DL:(ht + 1) * DL],
                            in_=dram[ht * 128:(ht + 1) * 128, :])
                    w_sb[name] = wt

                # --- q^T / k^T, blocked over s for AG pipelining ---
                with tc.tile_pool(name="pjqk", bufs=6, space="PSUM") \
                        as pjqk_psum, \
                     tc.tile_pool(name="pjst", bufs=6) as pj_stage:
                    for blk in range(NSPL):
                        for qk, bname, wname in ((0, "bq", "wq"),
                                                 (1, "bk", "wk")):
                            wt = w_sb[wname]
                            for dd in range(nDL):
                                pss = []
                                for _ssl in range(nSSb):
                                    ps_t = pjqk_psum.tile([128, 512], FP32,
                                                          tag="ps")
                                    pss.append(ps_t)
                                for ht in range(nH):
                                    wsl = wt[:, ht * DL + dd * 128:
                                             ht * DL + dd * 128 + 128]
                                    for ssl in range(nSSb):
                                        nc.tensor.matmul(
                                            pss[ssl][:],
                                            wsl,
                                            xt_sb[:, ht * S + blk * Sn
                                                  + ssl * 512:
                                                  ht * S + blk * Sn
                                                  + ssl * 512 + 512],
                                            start=(ht == 0),
                                            stop=(ht == nH - 1))
                                for ssl in range(nSSb):
                                    st = pj_stage.tile([128, 512], BF16,
                                                       tag="st")
                                    if with_bias:
                                        nc.scalar.activation(
                                            st[:], pss[ssl][:], ACT_COPY,
                                            bias=bias_vec[bname][:,
                                                                 dd:dd + 1])
                                    else:
                                        nc.scalar.activation(
                                            st[:], pss[ssl][:], ACT_COPY)
                                    nc.sync.dma_start(
                                        out=qk_my[blk][qk,
                                                       dd * 128:(dd + 1) * 128,
                                                       ssl * 512:ssl * 512 + 512],
                                        in_=st[:])
                        # one all-gather per block covers q^T and k^T
                        nc.gpsimd.collective_compute(
                            "AllGather", mybir.AluOpType.bypass,
                            ins=[qk_my[blk][:]], outs=[qk_ag[blk][:]],
                            replica_groups=groups)

                # --- v / g (x-stationary, w-moving), full S ---
                with tc.tile_pool(name="pjv", bufs=2, space="PSUM") as pv_ps, \
                     tc.tile_pool(name="pjg", bufs=2, space="PSUM") as pg_ps:
                    for st_i in range(S // 128):
                        ps_v = pv_ps.tile([128, DL], FP32, tag="psv")
                        ps_g = pg_ps.tile([128, DL], FP32, tag="psg")
                        for ht in range(nH):
                            xsl = xt_sb[:, ht * S + st_i * 128:
                                        ht * S + st_i * 128 + 128]
                            nc.tensor.matmul(
                                ps_v[:], xsl,
                                w_sb["wv"][:, ht * DL:(ht + 1) * DL],
                                start=(ht == 0),
                                stop=(not with_bias and ht == nH - 1))
                            nc.tensor.matmul(
                                ps_g[:], xsl,
                                w_sb["wg"][:, ht * DL:(ht + 1) * DL],
                                start=(ht == 0),
                                stop=(not with_bias and ht == nH - 1))
                        if with_bias:
                            nc.tensor.matmul(
                                ps_v[:], ones_row[0:1, 0:128],
                                bias_row["bv"][0:1, :],
                                start=False, stop=True)
                            nc.tensor.matmul(
                                ps_g[:], ones_row[0:1, 0:128],
                                bias_row["bg"][0:1, :],
                                start=False, stop=True)
                        nc.scalar.activation(
                            v_sb[:, st_i * DL:(st_i + 1) * DL], ps_v[:],
                            ACT_COPY)
                        nc.scalar.activation(
                            g_sb[:, st_i * DL:(st_i + 1) * DL], ps_g[:],
                            ACT_SIGMOID)

            # ---------- phase 2: chunk loop ----------
            with tc.tile_pool(name="chq", bufs=2) as chq_pool, \
                 tc.tile_pool(name="chk", bufs=2) as chk_pool, \
                 tc.tile_pool(name="chn", bufs=3) as chn_pool, \
                 tc.tile_pool(name="sm", bufs=3) as sm_pool, \
                 tc.tile_pool(name="ysb", bufs=2) as ysb_pool, \
                 tc.tile_pool(name="dstg", bufs=18) as dstg_pool, \
                 tc.tile_pool(name="ostage", bufs=3) as ostage_pool, \
                 tc.tile_pool(name="sc_ps", bufs=1, space="PSUM") as sc_ps_pool, \
                 tc.tile_pool(name="y_ps", bufs=2, space="PSUM") as y_ps_pool, \
                 tc.tile_pool(name="d_ps", bufs=2, space="PSUM") as d_ps_pool, \
                 tc.tile_pool(name="t_ps", bufs=1, space="PSUM") as t_ps_pool, \
                 tc.tile_pool(name="o_ps", bufs=1, space="PSUM") as o_ps_pool:

                q2 = k2 = None
                knt_prev = None
                pend_dps = pend_stages = None

                for i in range(nC):
                    blk, s0 = i // cpb, (i % cpb) * cs
                    half = (i % 2) * cs

                    # --- input tiles: q^T/k^T loaded per PAIR of chunks ---
                    if i % 2 == 0:
                        q2 = chq_pool.tile([128, nDT * 2 * cs], BF16,
                                           tag="q2")
                        k2 = chk_pool.tile([128, nDT * 2 * cs], BF16,
                                           tag="k2")
                        for r in range(group):
                            for qk, dst in ((0, q2), (1, k2)):
                                nc.sync.dma_start(
                                    out=dst[:, r * nDL * 2 * cs:
                                            (r + 1) * nDL * 2 * cs].rearrange(
                                        "p (t s) -> p t s", s=2 * cs),
                                    in_=qk_ag[blk][r, qk].rearrange(
                                        "(t p) s -> p t s",
                                        p=128)[:, :, s0:s0 + 2 * cs])

                    def qT(t):
                        return q2[:, t * 2 * cs + half:t * 2 * cs + half + cs]

                    def kT(t, h=None):
                        hh = half if h is None else h
                        return k2[:, t * 2 * cs + hh:t * 2 * cs + hh + cs]

                    # k natural via DMA transpose: [s, dk] tiles
                    knt = chn_pool.tile([128, nDT * 128], BF16, tag="knt")
                    for r in range(group):
                        nc.sync.dma_start(
                            out=knt[:, r * DL:(r + 1) * DL],
                            in_=qk_ag[blk][r, 1][:, s0:s0 + cs],
                            transpose=True)
                    vc = v_sb[:, i * DL:(i + 1) * DL]
                    gc = g_sb[:, i * DL:(i + 1) * DL]

                    # --- scores first (softmax overlaps cross matmuls) ---
                    sc = sc_ps_pool.tile([128, 128], FP32, tag="sc")
                    for t in range(nDT):
                        nc.tensor.matmul(sc[:], qT(t), kT(t),
                                         start=(t == 0), stop=(t == nDT - 1))

                    if i % 2 == 1:
                        # A^T = K_{i-1} Q_i^T for the exact rank-128 pair
                        # correction (stationary = prev-chunk k^T tiles)
                        at_ps = sc_ps_pool.tile([128, 128], FP32, tag="at")
                        for t in range(nDT):
                            nc.tensor.matmul(
                                at_ps[:], kT(t, 0), qT(t),
                                start=(t == 0), stop=(t == nDT - 1))
                        # pair update d_ps[t] = knt_prev[t]^T v_prev
                        #                      + knt[t]^T v  -> fp16 stages
                        pvc = v_sb[:, (i - 1) * DL:i * DL]
                        pend_dps = []
                        pend_stages = []
                        for t in range(nDT):
                            dps = d_ps_pool.tile([128, DL], FP32, tag="dps")
                            nc.tensor.matmul(
                                dps[:], knt_prev[:, t * 128:(t + 1) * 128],
                                pvc[:], start=True, stop=False)
                            nc.tensor.matmul(
                                dps[:], knt[:, t * 128:(t + 1) * 128],
                                vc[:], start=False, stop=True)
                            dstg = dstg_pool.tile([128, DL], FP16,
                                                  tag="dstg")
                            pend_dps.append(dps)
                            pend_stages.append(dstg)

                    # --- softmax (DVE + Act) ---
                    masked = sm_pool.tile([128, 128], FP32, tag="masked")
                    nc.vector.tensor_add(masked[:], sc[:], causal[:])
                    probs = sm_pool.tile([128, 128], BF16, tag="probs")
                    denom = sm_pool.tile([128, 1], FP32, tag="denom")
                    nc.scalar.activation(probs[:], masked[:], ACT_EXP,
                                         scale=scale, accum_out=denom[:])
                    rden = sm_pool.tile([128, 1], FP32, tag="rden")
                    nc.vector.reciprocal(rden[:], denom[:])
                    probsn = sm_pool.tile([128, 128], BF16, tag="probsn")
                    nc.vector.tensor_scalar_mul(probsn[:], probs[:], rden[:])

                    if i % 2 == 1:
                        # Act stages (after exp so it doesn't delay softmax)
                        for t in range(nDT):
                            nc.scalar.activation(pend_stages[t][:],
                                                 pend_dps[t][:], ACT_COPY)

                    if i % 2 == 0 and i >= 2:
                        # master += pair (i-2,i-1), DVE, t-ordered so the
                        # cross matmuls below pipeline behind them
                        for t in range(nDT):
                            nc.vector.tensor_add(
                                master[:, t * DL:(t + 1) * DL],
                                master[:, t * DL:(t + 1) * DL],
                                pend_stages[t][:])

                    # --- cross from the fp16 master (exact through pair
                    #     (i-2)//2; odd chunks add the A^T correction) ---
                    y_ps = y_ps_pool.tile([128, DL], FP32, tag="yps")
                    first = True
                    if i > 1:
                        for t in range(nDT):
                            nc.tensor.matmul(
                                y_ps[:], qT(t),
                                master[:, t * DL:(t + 1) * DL],
                                start=first, stop=False)
                            first = False

                    # probs transpose on PE (identity stationary)
                    pt_ps = t_ps_pool.tile([128, 512], BF16, tag="tps")
                    nc.tensor.transpose(pt_ps[:, 0:128], probsn[:], ident[:])
                    pt = sm_pool.tile([128, 128], BF16, tag="pt")
                    nc.vector.tensor_copy(pt[:], pt_ps[:, 0:128])

                    if i % 2 == 1:
                        at_sb = sm_pool.tile([128, 128], BF16, tag="atsb")
                        nc.vector.tensor_copy(at_sb[:], at_ps[:])
                        pvc = v_sb[:, (i - 1) * DL:i * DL]
                        nc.tensor.matmul(y_ps[:], at_sb[:], pvc[:],
                                         start=first, stop=False)
                        first = False

                    # local lands last in the y psum group
                    nc.tensor.matmul(y_ps[:], pt[:], vc[:], start=first,
                                     stop=True)

                    # --- gate + transpose y ---
                    y_sb = ysb_pool.tile([128, DL], BF16, tag="ysb")
                    nc.vector.tensor_mul(y_sb[:], y_ps[:], gc)
                    yt_ps = t_ps_pool.tile([128, 512], BF16, tag="tps")
                    for c4 in range(nDL):
                        nc.tensor.transpose(
                            yt_ps[:, c4 * 128:(c4 + 1) * 128],
                            y_sb[:, c4 * 128:(c4 + 1) * 128], ident[:])
                    yt = ysb_pool.tile([128, DL], BF16, tag="yt")
                    nc.vector.tensor_copy(yt[:], yt_ps[:, 0:DL])

                    # --- out projection partial ---
                    o_sb = ostage_pool.tile([128, H], FP16, tag="osb")
                    for hh in range(H // 512):
                        o_ps = o_ps_pool.tile([128, 512], FP32, tag="ops")
                        for t in range(nDL):
                            nc.tensor.matmul(
                                o_ps[:],
                                yt[:, t * 128:(t + 1) * 128],
                                wo_sb[:, t * H + hh * 512:
                                      t * H + hh * 512 + 512],
                                start=(t == 0),
                                stop=(not with_bias and t == nDL - 1))
                        if with_bias:
                            nc.tensor.matmul(
                                o_ps[:],
                                ones_row[0:1, 0:128],
                                bias_row["bo"][0:1, hh * 512:hh * 512 + 512],
                                start=False, stop=True)
                        nc.scalar.activation(
                            o_sb[:, hh * 512:(hh + 1) * 512], o_ps[:],
                            ACT_COPY)
                    ri, rl = i // cpr, i % cpr
                    nc.sync.dma_start(
                        out=part_out[ri][rl * cs:(rl + 1) * cs, :],
                        in_=o_sb[:])

                    # reduce-scatter, then DRAM->DRAM copy to the output
                    # (gpsimd queue: sits right behind its own RS)
                    if (i + 1) % cpr == 0:
                        r = i // cpr
                        nc.gpsimd.collective_compute(
                            "ReduceScatter", mybir.AluOpType.add,
                            ins=[part_out[r][:]],
                            outs=[rs_out[r][:]], replica_groups=groups)
                        nc.gpsimd.dma_start(out=y_out[r], in_=rs_out[r][:])

                    knt_prev = knt

    if split_waits:
        split_excess_waits(nc)
    return nc


def _prep_inputs(hidden_states, Wq, bq, Wk, bk, Wv, bv, Wg, bg, Wo, bo,
                 ncore=NCORE, group=GROUP):
    import ml_dtypes
    bf16 = ml_dtypes.bfloat16
    D_ = Wq.shape[1]
    DL = D_ // group
    hidden_states = np.asarray(hidden_states, np.float32)
    in_maps = []
    for c in range(ncore):
        b, j = c // group, c % group
        sl = slice(j * DL, (j + 1) * DL)
        in_maps.append({
            "xt": np.ascontiguousarray(hidden_states[b].T).astype(bf16),
            "wq": np.ascontiguousarray(
                np.asarray(Wq, np.float32)[:, sl]).astype(bf16),
            "wk": np.ascontiguousarray(
                np.asarray(Wk, np.float32)[:, sl]).astype(bf16),
            "wv": np.ascontiguousarray(
                np.asarray(Wv, np.float32)[:, sl]).astype(bf16),
            "wg": np.ascontiguousarray(
                np.asarray(Wg, np.float32)[:, sl]).astype(bf16),
            "wo": np.ascontiguousarray(
                np.asarray(Wo, np.float32)[sl, :]).astype(bf16),
            "bq": np.ascontiguousarray(np.asarray(bq, np.float32)[sl]),
            "bk": np.ascontiguousarray(np.asarray(bk, np.float32)[sl]),
            "bv": np.ascontiguousarray(np.asarray(bv, np.float32)[sl]),
            "bg": np.ascontiguousarray(np.asarray(bg, np.float32)[sl]),
            "bo": (np.asarray(bo, np.float32) / group),
        })
    return in_maps


def _assemble(results, B=B, S=S, H=H, group=GROUP, rspl=None):
    if rspl is None:
        import os
        rspl = int(os.environ.get("KERNEL_RSPL", "4"))
    Sn2 = S // rspl
    Sg2 = Sn2 // group
    out = np.empty((B, S, H), np.float32)
    for b in range(B):
        for j in range(group):
            y = np.asarray(results[b * group + j]["y_red"], np.float32)
            for r in range(rspl):
                out[b, r * Sn2 + j * Sg2: r * Sn2 + (j + 1) * Sg2] = y[r]
    return out


_NC_CACHE = {}


def get_program(with_bias=False):
    import os
    nspl = int(os.environ.get("KERNEL_NSPL", "2"))
    rspl = int(os.environ.get("KERNEL_RSPL", "4"))
    key = (B, S, H, D, nspl, rspl, with_bias)
    if key not in _NC_CACHE:
        _NC_CACHE[key] = build(nspl=nspl, rspl=rspl, with_bias=with_bias)
    return _NC_CACHE[key]


def kernel(hidden_states, Wq, bq, Wk, bk, Wv, bv, Wg, bg, Wo, bo):
    with_bias = any(
        np.any(np.asarray(b)) for b in (bq, bk, bv, bg, bo))
    nc = get_program(with_bias=with_bias)
    in_maps = _prep_inputs(hidden_states, Wq, bq, Wk, bk, Wv, bv, Wg, bg,
                           Wo, bo)
    res = run_bass_kernel_spmd(nc, in_maps, list(range(NCORE)))
    return _assemble(res.results)
